# revision 30
# baseline (speedup 1.0000x reference)
"""Trainium2 Bass kernel for nn_DataReuploadingEncoder (4-qubit data
re-uploading circuit, B=1048576 samples, 8-core data parallel).

Complex-real ("L2") layout: state partition index p = 64*c + 16*g + j with
c in {re=0, im=1}, g sample-group, j state index; free dim = samples.  Each
fixed 16x16 complex gate is ONE 128x128 real stationary (4 diagonal 32x32
complex-real blocks [[Wr,-Wi],[Wi,Wr]]).

Per layer:  s = D1 s ; s = M_mid s ; s = D2 s ; s = M_l s   where the D's
are per-sample diagonals exp(i*phi).  Applying D then M is computed as
    P_next = M . (cos (.) P)  +  Msw . (sin (.) P)
two accumulating matmuls, where Msw is M with its input partition halves
swapped: this folds the re/im cross-term of the diagonal into the
stationary (compute lanes are partition-locked and cannot cross the re/im
halves).  The +/- signs of the sin terms are FOLDED INTO THE TRIG TILES:
the phase matmul constants negate phi on im rows (phi' = (-1)^c phi_j).

Layer-0's D1 acts on the uniform state: its output is the full-angle trig
tile directly (folded sign on im rows), compensated by a column-flipped
copy of the mid gate (W0b).  The finish uses the squared state as the
matmul STATIONARY and the sign matrix as moving operand, so the output
lands with samples in partitions (no output transposes).

Tiles are processed in PAIRS with instruction-level interleaving of the
two chains, so every engine queue alternates between two independent
dependency chains.

Sample mapping per core (bs = 131072):
  flat load: fl[p, n] = x[1024 p + n//4, n%4],  transpose+tanh ->
  th_all[r=4*sl+i, b, p] = tanh(x[1024 p + 32 b + sl, i])
  tile T (2048 samples): b = T//2, parity h = T%2;
  free col f = 128 k + p (k in 0..3);  group g:  sl = 16 h + 4 k + g.
"""

import numpy as np

N_QUBITS = 4
N_LAYERS = 3
DIM = 16
G4 = 4          # sample groups per tile (partition packing)
FCOL = 512      # samples per group per tile -> 2048 samples per tile
N_CORES = 8

# ----------------------------------------------------------------------------
# host-side constant construction
# ----------------------------------------------------------------------------


def _rz(t):
    return np.diag([np.exp(-0.5j * t), np.exp(0.5j * t)]).astype(np.complex128)


def _ry(t):
    c, s = np.cos(t / 2), np.sin(t / 2)
    return np.array([[c, -s], [s, c]], dtype=np.complex128)


def _rot(phi, theta, omega):
    return _rz(omega) @ _ry(theta) @ _rz(phi)


def _kron4(mats):
    out = mats[0]
    for m in mats[1:]:
        out = np.kron(out, m)
    return out


def _cnot_mat(c, t):
    P = np.zeros((DIM, DIM), dtype=np.complex128)
    for j in range(DIM):
        bc = (j >> (3 - c)) & 1
        jj = j ^ (1 << (3 - t)) if bc else j
        P[jj, j] = 1.0
    return P


def _bit(j, i):
    return (j >> (3 - i)) & 1


def _build_constants(weights, scaling):
    weights = np.asarray(weights, dtype=np.float64)
    scaling = np.asarray(scaling, dtype=np.float64)

    A = np.zeros((N_LAYERS, N_QUBITS, DIM))
    for l in range(N_LAYERS):
        for i in range(N_QUBITS):
            for j in range(DIM):
                sgn = 1.0 if _bit(j, i) else -1.0
                A[l, i, j] = sgn * np.pi * scaling[l, i] / 2.0

    # dedup identical scaling rows (harness uses all-ones -> u == 1)
    uniq = []
    lmap = []
    for l in range(N_LAYERS):
        for k, ku in enumerate(uniq):
            if np.array_equal(A[l], A[ku]):
                lmap.append(k)
                break
        else:
            uniq.append(l)
            lmap.append(len(uniq) - 1)
    A_u = A[uniq]  # [u, 4, 16]

    S = np.diag([1.0, 1.0j]).astype(np.complex128)
    H = np.array([[1, 1], [1, -1]], dtype=np.complex128) / np.sqrt(2.0)
    SH = S @ H
    HSd = H @ S.conj().T

    C = np.eye(DIM, dtype=np.complex128)
    for i in range(N_QUBITS):
        C = _cnot_mat(i, (i + 1) % N_QUBITS) @ C

    F_SH = _kron4([SH] * 4)
    F_HS = _kron4([HSd] * 4)
    R = [_kron4([_rot(*weights[l, i]) for i in range(N_QUBITS)])
         for l in range(N_LAYERS)]

    gates = [F_SH,
             0.25 * (F_HS @ C @ R[0]),
             F_HS @ C @ R[1],
             C @ R[2]]

    sign = np.zeros((DIM, N_QUBITS))
    for j in range(DIM):
        for w in range(N_QUBITS):
            sign[j, w] = 1.0 - 2.0 * _bit(j, w)

    return A_u, lmap, gates, sign


def _cplx_block(M):
    """complex 16x16 -> real 32x32 on (c,j) vectors: [[Wr,-Wi],[Wi,Wr]]."""
    Wr, Wi = np.real(M), np.imag(M)
    B = np.zeros((32, 32))
    B[:16, :16] = Wr
    B[:16, 16:] = -Wi
    B[16:, :16] = Wi
    B[16:, 16:] = Wr
    return B


def _host_tensors(weights, scaling, dt_state=np.float16):
    A_u, lmap, gates, sign = _build_constants(weights, scaling)
    u = A_u.shape[0]

    # phase-matmul stationaries: phim[4h+k][r=4*sl+i, l, P=64c+16g+j]
    #   = (-1)^c * A_u[l,i,j] * [sl == 16h+4k+g]   (sign folding on im rows)
    phim = np.zeros((8, 128, u, 128), dtype=np.float64)
    for h in range(2):
        for k in range(4):
            for g in range(G4):
                sl = 16 * h + 4 * k + g
                for i in range(N_QUBITS):
                    r = 4 * sl + i
                    for l in range(u):
                        for c in range(2):
                            sgnc = 1.0 if c == 0 else -1.0
                            base = 64 * c + 16 * g
                            phim[4 * h + k, r, l, base:base + 16] = \
                                sgnc * A_u[l, i]
    phim = phim.astype(dt_state)

    # gate stationaries: [W0b, W0, G1, G2, G3]; lhsT[p_in, 2*gi, p_out] =
    # block[p_out_local, p_in_local] replicated over the 4 groups; 2*gi+1
    # is the input-half-SWAPPED copy (rows 0:64 <-> 64:128).
    blocks = []
    B0 = _cplx_block(gates[0])
    B0b = B0.copy()
    B0b[:, 16:] *= -1.0   # compensate folded (-sin) im rows of v0
    blocks.append(B0b)
    blocks.append(B0)
    for gi in range(1, 4):
        blocks.append(_cplx_block(gates[gi]))

    wm = np.zeros((128, 10, 128), dtype=np.float64)
    for gi, B in enumerate(blocks):
        lt = np.zeros((128, 128))
        for g in range(G4):
            for ci in range(2):
                for j in range(DIM):
                    p_in = 64 * ci + 16 * g + j
                    for co in range(2):
                        for j2 in range(DIM):
                            p_out = 64 * co + 16 * g + j2
                            lt[p_in, p_out] = B[16 * co + j2, 16 * ci + j]
        wm[:, 2 * gi, :] = lt
        perm = np.arange(128) ^ 64
        wm[:, 2 * gi + 1, :] = lt[perm, :]
    wm = wm.astype(dt_state)

    # sign-reduce MOVING operand: sgm2[64c+16g+j, 4g+w] = sign[j, w]
    sgm2 = np.zeros((128, 16), dtype=np.float64)
    for c in range(2):
        for g in range(G4):
            for j in range(DIM):
                sgm2[64 * c + 16 * g + j, 4 * g:4 * g + 4] = sign[j]
    sgm2 = sgm2.astype(dt_state)

    id128 = np.eye(128, dtype=np.float32)

    return {"phimats": phim, "wmats": wm, "signmat": sgm2,
            "id128": id128}, u, lmap


# ----------------------------------------------------------------------------
# bass kernel
# ----------------------------------------------------------------------------

_NC_CACHE = {}

DT_STATE = "float16"


def _build_nc(bs, u, lmap, dt_state_name=None):
    import concourse.tile as tile
    from concourse import bacc, mybir
    from contextlib import ExitStack

    f32 = mybir.dt.float32
    dt_st = getattr(mybir.dt, dt_state_name or DT_STATE)
    ACT = mybir.ActivationFunctionType
    MULT = mybir.AluOpType.mult

    ntiles = bs // (G4 * FCOL)
    assert bs % (G4 * FCOL) == 0
    assert ntiles % 2 == 0
    npairs = ntiles // 2
    nflat = bs * N_QUBITS // 128          # elements per partition, flat load
    nblk = nflat // 128
    assert nblk == ntiles // 2
    assert u <= 2

    nc = bacc.Bacc("TRN2", target_bir_lowering=False, debug=False)
    x_ap = nc.dram_tensor("x", [bs, N_QUBITS], f32, kind="ExternalInput").ap()
    phim_ap = nc.dram_tensor("phimats", [8, 128, u, 128], dt_st,
                             kind="ExternalInput").ap()
    wm_ap = nc.dram_tensor("wmats", [128, 10, 128], dt_st,
                           kind="ExternalInput").ap()
    sg_ap = nc.dram_tensor("signmat", [128, 16], dt_st,
                           kind="ExternalInput").ap()
    id128_ap = nc.dram_tensor("id128", [128, 128], f32,
                              kind="ExternalInput").ap()
    out_ap = nc.dram_tensor("out", [bs, N_QUBITS], f32,
                            kind="ExternalOutput").ap()

    halfpi = nc.alloc_sbuf_tensor("halfpi", [128, 1], f32)
    nc.gpsimd.memset(halfpi.ap(), float(np.pi / 2))
    nc.all_engine_barrier()

    with tile.TileContext(nc) as tc:
        with ExitStack() as ctx:
            consts = ctx.enter_context(tc.tile_pool(name="consts", bufs=1))
            bigp = ctx.enter_context(tc.tile_pool(name="big", bufs=1))
            trigp = ctx.enter_context(tc.tile_pool(name="trig", bufs=4))
            tmpp = ctx.enter_context(tc.tile_pool(name="tmp", bufs=4))
            sqp = ctx.enter_context(tc.tile_pool(name="sq", bufs=2))
            # PSUM: P1 3x2KB + P2 2x2KB + phi 2x2KB + zo 2x0.25KB = 14.5KB
            phip = ctx.enter_context(tc.tile_pool(name="phip", bufs=2,
                                                  space="PSUM"))
            gpA = ctx.enter_context(tc.tile_pool(name="gpA", bufs=3,
                                                 space="PSUM"))
            gpB = ctx.enter_context(tc.tile_pool(name="gpB", bufs=2,
                                                 space="PSUM"))
            zop = ctx.enter_context(tc.tile_pool(name="zop", bufs=1,
                                                 space="PSUM"))

            # constants
            phim = consts.tile([128, 8, u, 128], dt_st)
            for k in range(8):
                nc.sync.dma_start(phim[:, k, :, :], phim_ap[k, :, :, :])
            wm = consts.tile([128, 10, 128], dt_st)
            nc.sync.dma_start(wm[:], wm_ap[:])
            sgm2 = consts.tile([128, 16], dt_st)
            nc.sync.dma_start(sgm2[:], sg_ap[:])
            id128 = consts.tile([128, 128], f32)
            nc.sync.dma_start(id128[:], id128_ap[:])

            # ---- input: contiguous load, transpose, tanh -----------------
            fl = bigp.tile([128, nflat], f32)
            xflat = x_ap[:].rearrange("(p s) i -> p (s i)", p=128)
            nc.sync.dma_start(fl[:], xflat)
            th_all = bigp.tile([128, nblk, 128], dt_st)
            for b in range(nblk):
                tp = gpA.tile([128, 128], f32, tag="P1")
                nc.tensor.transpose(tp[:], fl[:, 128 * b:128 * (b + 1)],
                                    id128[:])
                nc.scalar.activation(th_all[:, b, :], tp[:], ACT.Tanh)

            # od_all free dims: (b, h, k, g, w) -> col 128b+64h+16k+4g+w
            od_all = bigp.tile([128, nblk, 2, 4, 16], f32)

            def emit_phi_trig(t, c):
                """phase matmuls + trig for tile t (chain c): returns
                ([per-l (cs1, cs2)], v0); trig tiles [128, 2, FCOL] fp16
                (0=cos, 1=sin; sin sign-folded: negated on im rows)."""
                b, h = t // 2, t % 2
                phi = phip.tile([128, u, FCOL], f32, tag="phi")
                for l in range(u):
                    for k in range(4):
                        nc.tensor.matmul(
                            phi[:, l, 128 * k:128 * (k + 1)],
                            phim[:, 4 * h + k, l, :], th_all[:, b, :],
                            start=True, stop=True)
                res = []
                v0 = None
                for l in range(u):
                    aphi = trigp.tile([128, FCOL], f32, tag="aphi", name=f"aphi{c}")
                    nc.scalar.activation(aphi[:], phi[:, l, :], ACT.Abs)
                    cs2 = trigp.tile([128, 2, FCOL], dt_st,
                                     tag="cs2", name=f"cs2{c}")
                    nc.scalar.activation(cs2[:, 1, :], phi[:, l, :], ACT.Sin,
                                         scale=0.5)
                    nc.scalar.activation(cs2[:, 0, :], aphi[:], ACT.Sin,
                                         bias=halfpi.ap(), scale=-0.5)
                    ssq = trigp.tile([128, FCOL], dt_st, tag="ssq", name=f"ssq{c}")
                    nc.vector.tensor_tensor(out=ssq[:], in0=cs2[:, 1, :],
                                            in1=cs2[:, 1, :], op=MULT)
                    cs1 = trigp.tile([128, 2, FCOL], dt_st,
                                     tag="cs1", name=f"cs1{c}")
                    # cos(phi) = 1 - 2 sin^2(phi/2)
                    nc.scalar.activation(cs1[:, 0, :], ssq[:], ACT.Copy,
                                         bias=1.0, scale=-2.0)
                    # sin(phi') = 2 sin(phi'/2) cos(phi'/2)
                    nc.vector.scalar_tensor_tensor(out=cs1[:, 1, :],
                                                   in0=cs2[:, 1, :],
                                                   scalar=2.0,
                                                   in1=cs2[:, 0, :],
                                                   op0=MULT, op1=MULT)
                    if l == lmap[0]:
                        v0 = trigp.tile([128, FCOL], dt_st,
                                        tag="v0", name=f"v0{c}")
                        nc.scalar.activation(v0[0:64, :], ssq[0:64, :],
                                             ACT.Copy, bias=1.0, scale=-2.0)
                        nc.vector.scalar_tensor_tensor(
                            out=v0[64:128, :], in0=cs2[64:128, 1, :],
                            scalar=2.0, in1=cs2[64:128, 0, :],
                            op0=MULT, op1=MULT)
                    res.append((cs1, cs2))
                return res, v0

            def dapply_pair(Ps, csts, split):
                """For each chain c: t1_c = cos (.) P_c, t2_c = sin (.) P_c.
                split=False: one broadcast multiply on Vector (reads PSUM
                twice).  split=True: downconvert P once on Scalar, then
                multiplies on Vector/GpSimd from fp16 SBUF."""
                outs = []
                if not split:
                    for c, (P, cst) in enumerate(zip(Ps, csts)):
                        tb = tmpp.tile([128, 2, FCOL], dt_st, tag=f"tb{c}")
                        prep = P[:].unsqueeze(1).to_broadcast((128, 2, FCOL))
                        nc.vector.tensor_tensor(out=tb[:], in0=csts[c][:],
                                                in1=prep, op=MULT)
                        outs.append((tb[:, 0, :], tb[:, 1, :]))
                    return outs
                Pcs = []
                for c, P in enumerate(Ps):
                    Pc = tmpp.tile([128, FCOL], dt_st, tag=f"pc{c}")
                    nc.scalar.activation(Pc[:], P[:], ACT.Copy, bias=0.0)
                    Pcs.append(Pc)
                t1s = []
                for c in range(2):
                    t1 = tmpp.tile([128, FCOL], dt_st, tag=f"t1{c}")
                    nc.vector.tensor_tensor(out=t1[:], in0=csts[c][:, 0, :],
                                            in1=Pcs[c][:], op=MULT)
                    t1s.append(t1)
                for c in range(2):
                    t2 = tmpp.tile([128, FCOL], dt_st, tag=f"t2{c}")
                    nc.gpsimd.tensor_tensor(out=t2[:], in0=csts[c][:, 1, :],
                                            in1=Pcs[c][:], op=MULT)
                    outs.append((t1s[c][:], t2[:]))
                return outs

            def gate_mm_pair(pool, tag, gi, tts):
                """P_c = W_gi . t1_c + W_gi_swapped . t2_c, W-adjacent."""
                Ps = [pool.tile([128, FCOL], f32, tag=tag,
                                name=f"{tag}_{gi}_{c}") for c in range(2)]
                for c in range(2):
                    nc.tensor.matmul(Ps[c][:], wm[:, 2 * gi, :], tts[c][0],
                                     start=True, stop=False)
                for c in range(2):
                    nc.tensor.matmul(Ps[c][:], wm[:, 2 * gi + 1, :],
                                     tts[c][1], start=False, stop=True)
                return Ps

            # ---- main loop: adjacent tile pairs, chains interleaved -----
            pair_list = [(2 * pr, 2 * pr + 1) for pr in range(npairs)]

            trigs = [emit_phi_trig(pair_list[0][c], c) for c in range(2)]
            for pr, ts in enumerate(pair_list):
                trig_next = None
                P2s = [None, None]
                for l in range(N_LAYERS):
                    li = lmap[l]
                    if l == 0:
                        P1s = []
                        for c in range(2):
                            P1 = gpA.tile([128, FCOL], f32, tag="P1")
                            nc.tensor.matmul(P1[:], wm[:, 0, :],
                                             trigs[c][1][:],
                                             start=True, stop=True)
                            P1s.append(P1)
                    else:
                        tts = dapply_pair(P2s, [trigs[c][0][li][0]
                                                for c in range(2)],
                                          split=False)
                        P1s = gate_mm_pair(gpA, "P1", 1, tts)
                    tts = dapply_pair(P1s, [trigs[c][0][li][1]
                                            for c in range(2)], split=True)
                    P2s = gate_mm_pair(gpB, "P2", 2 + l, tts)
                    if l == 0 and pr + 1 < npairs:
                        trig_next = [emit_phi_trig(pair_list[pr + 1][c], c)
                                     for c in range(2)]
                # ---- finish: square, sign-reduce (state as stationary) --
                for c in range(2):
                    t = ts[c]
                    b, h = t // 2, t % 2
                    sq = sqp.tile([128, FCOL], dt_st, tag=f"sq{c}")
                    nc.scalar.activation(sq[:], P2s[c][:], ACT.Square)
                    zo = zop.tile([128, 4, 16], f32, tag="zo")
                    for k in range(4):
                        nc.tensor.matmul(zo[:, k, :],
                                         sq[:, 128 * k:128 * (k + 1)],
                                         sgm2[:], start=True, stop=True)
                    nc.scalar.copy(od_all[:, b, h, :, :], zo[:])
                if trig_next is not None:
                    trigs = trig_next

            # ---- final store --------------------------------------------
            oflat = out_ap[:].rearrange("(p s) w -> p (s w)", p=128)
            nc.sync.dma_start(oflat,
                              od_all[:].rearrange("p a b c d -> p (a b c d)"))

    nc.compile()
    return nc


def _get_nc(bs, u, lmap, dt_state_name=None):
    key = (bs, u, tuple(lmap), dt_state_name or DT_STATE)
    if key not in _NC_CACHE:
        _NC_CACHE[key] = _build_nc(bs, u, lmap, dt_state_name)
    return _NC_CACHE[key]


def _np_dt(name):
    import ml_dtypes
    return {"float32": np.float32, "float32r": np.float32,
            "float16": np.float16, "bfloat16": ml_dtypes.bfloat16}[name]


def kernel(x, weights, scaling):
    from concourse.bass_utils import run_bass_kernel_spmd

    x = np.ascontiguousarray(np.asarray(x, dtype=np.float32))
    B = x.shape[0]
    consts, u, lmap = _host_tensors(weights, scaling,
                                    dt_state=_np_dt(DT_STATE))

    chunk = N_CORES * G4 * FCOL * 2
    Bp = ((B + chunk - 1) // chunk) * chunk
    if Bp != B:
        xp = np.zeros((Bp, x.shape[1]), dtype=np.float32)
        xp[:B] = x
        x = xp
    bs = Bp // N_CORES

    nc = _get_nc(bs, u, lmap)
    xs = x.reshape(N_CORES, bs, x.shape[1])
    in_maps = [dict(consts, x=np.ascontiguousarray(xs[i]))
               for i in range(N_CORES)]
    res = run_bass_kernel_spmd(nc, in_maps, core_ids=list(range(N_CORES)))
    out = np.concatenate([r["out"] for r in res.results], axis=0)
    return out[:B]


# revision 31
# speedup vs baseline: 1.2613x; 1.2613x over previous
"""Trainium2 Bass kernel for nn_DataReuploadingEncoder (4-qubit data
re-uploading circuit, B=1048576 samples, 8-core data parallel).

Complex-real ("L2") layout: state partition index p = 64*c + 16*g + j with
c in {re=0, im=1}, g sample-group, j state index; free dim = samples.  Each
fixed 16x16 complex gate is ONE 128x128 real stationary (4 diagonal 32x32
complex-real blocks [[Wr,-Wi],[Wi,Wr]]).

Per layer:  s = D1 s ; s = M_mid s ; s = D2 s ; s = M_l s   where the D's
are per-sample diagonals exp(i*phi).  Applying D then M is computed as
    P_next = M . (cos (.) P)  +  Msw . (sin (.) P)
two accumulating matmuls, where Msw is M with its input partition halves
swapped: this folds the re/im cross-term of the diagonal into the
stationary (compute lanes are partition-locked and cannot cross the re/im
halves).  The +/- signs of the sin terms are FOLDED INTO THE TRIG TILES:
the phase matmul constants negate phi on im rows (phi' = (-1)^c phi_j).

Layer-0's D1 acts on the uniform state: its output is the full-angle trig
tile directly (folded sign on im rows), compensated by a column-flipped
copy of the mid gate (W0b).  The finish uses the squared state as the
matmul STATIONARY and the sign matrix as moving operand, so the output
lands with samples in partitions (no output transposes).

Tiles are processed in PAIRS with instruction-level interleaving of the
two chains, so every engine queue alternates between two independent
dependency chains.

Sample mapping per core (bs = 131072):
  flat load: fl[p, n] = x[1024 p + n//4, n%4],  transpose+tanh ->
  th_all[r=4*sl+i, b, p] = tanh(x[1024 p + 32 b + sl, i])
  tile T (2048 samples): b = T//2, parity h = T%2;
  free col f = 128 k + p (k in 0..3);  group g:  sl = 16 h + 4 k + g.
"""

import numpy as np

N_QUBITS = 4
N_LAYERS = 3
DIM = 16
G4 = 4          # sample groups per tile (partition packing)
FCOL = 512      # samples per group per tile -> 2048 samples per tile
N_CORES = 8

# ----------------------------------------------------------------------------
# host-side constant construction
# ----------------------------------------------------------------------------


def _rz(t):
    return np.diag([np.exp(-0.5j * t), np.exp(0.5j * t)]).astype(np.complex128)


def _ry(t):
    c, s = np.cos(t / 2), np.sin(t / 2)
    return np.array([[c, -s], [s, c]], dtype=np.complex128)


def _rot(phi, theta, omega):
    return _rz(omega) @ _ry(theta) @ _rz(phi)


def _kron4(mats):
    out = mats[0]
    for m in mats[1:]:
        out = np.kron(out, m)
    return out


def _cnot_mat(c, t):
    P = np.zeros((DIM, DIM), dtype=np.complex128)
    for j in range(DIM):
        bc = (j >> (3 - c)) & 1
        jj = j ^ (1 << (3 - t)) if bc else j
        P[jj, j] = 1.0
    return P


def _bit(j, i):
    return (j >> (3 - i)) & 1


def _build_constants(weights, scaling):
    weights = np.asarray(weights, dtype=np.float64)
    scaling = np.asarray(scaling, dtype=np.float64)

    A = np.zeros((N_LAYERS, N_QUBITS, DIM))
    for l in range(N_LAYERS):
        for i in range(N_QUBITS):
            for j in range(DIM):
                sgn = 1.0 if _bit(j, i) else -1.0
                A[l, i, j] = sgn * np.pi * scaling[l, i] / 2.0

    # dedup identical scaling rows (harness uses all-ones -> u == 1)
    uniq = []
    lmap = []
    for l in range(N_LAYERS):
        for k, ku in enumerate(uniq):
            if np.array_equal(A[l], A[ku]):
                lmap.append(k)
                break
        else:
            uniq.append(l)
            lmap.append(len(uniq) - 1)
    A_u = A[uniq]  # [u, 4, 16]

    S = np.diag([1.0, 1.0j]).astype(np.complex128)
    H = np.array([[1, 1], [1, -1]], dtype=np.complex128) / np.sqrt(2.0)
    SH = S @ H
    HSd = H @ S.conj().T

    C = np.eye(DIM, dtype=np.complex128)
    for i in range(N_QUBITS):
        C = _cnot_mat(i, (i + 1) % N_QUBITS) @ C

    F_SH = _kron4([SH] * 4)
    F_HS = _kron4([HSd] * 4)
    R = [_kron4([_rot(*weights[l, i]) for i in range(N_QUBITS)])
         for l in range(N_LAYERS)]

    gates = [F_SH,
             0.25 * (F_HS @ C @ R[0]),
             F_HS @ C @ R[1],
             C @ R[2]]

    sign = np.zeros((DIM, N_QUBITS))
    for j in range(DIM):
        for w in range(N_QUBITS):
            sign[j, w] = 1.0 - 2.0 * _bit(j, w)

    return A_u, lmap, gates, sign


def _cplx_block(M):
    """complex 16x16 -> real 32x32 on (c,j) vectors: [[Wr,-Wi],[Wi,Wr]]."""
    Wr, Wi = np.real(M), np.imag(M)
    B = np.zeros((32, 32))
    B[:16, :16] = Wr
    B[:16, 16:] = -Wi
    B[16:, :16] = Wi
    B[16:, 16:] = Wr
    return B


def _host_tensors(weights, scaling, dt_state=np.float16):
    A_u, lmap, gates, sign = _build_constants(weights, scaling)
    u = A_u.shape[0]

    # phase-matmul stationaries: phim[4h+k][r=4*sl+i, l, P=64c+16g+j]
    #   = (-1)^c * A_u[l,i,j] * [sl == 16h+4k+g]   (sign folding on im rows)
    phim = np.zeros((8, 128, u, 128), dtype=np.float64)
    for h in range(2):
        for k in range(4):
            for g in range(G4):
                sl = 16 * h + 4 * k + g
                for i in range(N_QUBITS):
                    r = 4 * sl + i
                    for l in range(u):
                        for c in range(2):
                            sgnc = 1.0 if c == 0 else -1.0
                            base = 64 * c + 16 * g
                            phim[4 * h + k, r, l, base:base + 16] = \
                                sgnc * A_u[l, i]
    phim = phim.astype(dt_state)

    # gate stationaries: [W0b, W0, G1, G2, G3]; lhsT[p_in, 2*gi, p_out] =
    # block[p_out_local, p_in_local] replicated over the 4 groups; 2*gi+1
    # is the input-half-SWAPPED copy (rows 0:64 <-> 64:128).
    blocks = []
    B0 = _cplx_block(gates[0])
    B0b = B0.copy()
    B0b[:, 16:] *= -1.0   # compensate folded (-sin) im rows of v0
    blocks.append(B0b)
    blocks.append(B0)
    for gi in range(1, 4):
        blocks.append(_cplx_block(gates[gi]))

    wm = np.zeros((128, 10, 128), dtype=np.float64)
    for gi, B in enumerate(blocks):
        lt = np.zeros((128, 128))
        for g in range(G4):
            for ci in range(2):
                for j in range(DIM):
                    p_in = 64 * ci + 16 * g + j
                    for co in range(2):
                        for j2 in range(DIM):
                            p_out = 64 * co + 16 * g + j2
                            lt[p_in, p_out] = B[16 * co + j2, 16 * ci + j]
        wm[:, 2 * gi, :] = lt
        perm = np.arange(128) ^ 64
        wm[:, 2 * gi + 1, :] = lt[perm, :]
    wm = wm.astype(dt_state)

    # sign-reduce MOVING operand: sgm2[64c+16g+j, 4g+w] = sign[j, w]
    sgm2 = np.zeros((128, 16), dtype=np.float64)
    for c in range(2):
        for g in range(G4):
            for j in range(DIM):
                sgm2[64 * c + 16 * g + j, 4 * g:4 * g + 4] = sign[j]
    sgm2 = sgm2.astype(dt_state)

    id128 = np.eye(128, dtype=np.float32)

    return {"phimats": phim, "wmats": wm, "signmat": sgm2,
            "id128": id128}, u, lmap


# ----------------------------------------------------------------------------
# bass kernel
# ----------------------------------------------------------------------------

_NC_CACHE = {}

DT_STATE = "float16"


def _build_nc(bs, u, lmap, dt_state_name=None):
    import concourse.tile as tile
    from concourse import bacc, mybir
    from contextlib import ExitStack

    f32 = mybir.dt.float32
    dt_st = getattr(mybir.dt, dt_state_name or DT_STATE)
    ACT = mybir.ActivationFunctionType
    MULT = mybir.AluOpType.mult

    ntiles = bs // (G4 * FCOL)
    assert bs % (G4 * FCOL) == 0
    assert ntiles % 2 == 0
    npairs = ntiles // 2
    nflat = bs * N_QUBITS // 128          # elements per partition, flat load
    nblk = nflat // 128
    assert nblk == ntiles // 2
    assert u <= 2

    nc = bacc.Bacc("TRN2", target_bir_lowering=False, debug=False)
    x_ap = nc.dram_tensor("x", [bs, N_QUBITS], f32, kind="ExternalInput").ap()
    phim_ap = nc.dram_tensor("phimats", [8, 128, u, 128], dt_st,
                             kind="ExternalInput").ap()
    wm_ap = nc.dram_tensor("wmats", [128, 10, 128], dt_st,
                           kind="ExternalInput").ap()
    sg_ap = nc.dram_tensor("signmat", [128, 16], dt_st,
                           kind="ExternalInput").ap()
    id128_ap = nc.dram_tensor("id128", [128, 128], f32,
                              kind="ExternalInput").ap()
    out_ap = nc.dram_tensor("out", [bs, N_QUBITS], f32,
                            kind="ExternalOutput").ap()

    halfpi = nc.alloc_sbuf_tensor("halfpi", [128, 1], f32)
    nc.gpsimd.memset(halfpi.ap(), float(np.pi / 2))
    nc.all_engine_barrier()

    with tile.TileContext(nc) as tc:
        with ExitStack() as ctx:
            consts = ctx.enter_context(tc.tile_pool(name="consts", bufs=1))
            bigp = ctx.enter_context(tc.tile_pool(name="big", bufs=1))
            trigp = ctx.enter_context(tc.tile_pool(name="trig", bufs=4))
            tmpp = ctx.enter_context(tc.tile_pool(name="tmp", bufs=4))
            sqp = ctx.enter_context(tc.tile_pool(name="sq", bufs=2))
            # PSUM: P1 3x2KB + P2 2x2KB + phi 2x2KB + zo 2x0.25KB = 14.5KB
            phip = ctx.enter_context(tc.tile_pool(name="phip", bufs=2,
                                                  space="PSUM"))
            gpA = ctx.enter_context(tc.tile_pool(name="gpA", bufs=3,
                                                 space="PSUM"))
            gpB = ctx.enter_context(tc.tile_pool(name="gpB", bufs=2,
                                                 space="PSUM"))
            zop = ctx.enter_context(tc.tile_pool(name="zop", bufs=1,
                                                 space="PSUM"))

            # constants
            phim = consts.tile([128, 8, u, 128], dt_st)
            for k in range(8):
                nc.sync.dma_start(phim[:, k, :, :], phim_ap[k, :, :, :])
            wm = consts.tile([128, 10, 128], dt_st)
            nc.sync.dma_start(wm[:], wm_ap[:])
            sgm2 = consts.tile([128, 16], dt_st)
            nc.sync.dma_start(sgm2[:], sg_ap[:])
            id128 = consts.tile([128, 128], f32)
            nc.sync.dma_start(id128[:], id128_ap[:])

            # ---- input: contiguous load, transpose, tanh -----------------
            fl = bigp.tile([128, nflat], f32)
            xflat = x_ap[:].rearrange("(p s) i -> p (s i)", p=128)
            nc.sync.dma_start(fl[:], xflat)
            th_all = bigp.tile([128, nblk, 128], dt_st)
            for b in range(nblk):
                tp = gpA.tile([128, 128], f32, tag="P1")
                nc.tensor.transpose(tp[:], fl[:, 128 * b:128 * (b + 1)],
                                    id128[:])
                nc.scalar.activation(th_all[:, b, :], tp[:], ACT.Tanh)

            # od_all free dims: (b, h, k, g, w) -> col 128b+64h+16k+4g+w
            od_all = bigp.tile([128, nblk, 2, 4, 16], f32)

            def emit_phi_trig(t, c):
                """phase matmuls + trig for tile t (chain c): returns
                ([per-l (cs1, cs2)], v0); trig tiles [128, 2, FCOL] fp16
                (0=cos, 1=sin; sin sign-folded: negated on im rows)."""
                b, h = t // 2, t % 2
                phi = phip.tile([128, u, FCOL], f32, tag="phi")
                for l in range(u):
                    for k in range(4):
                        nc.tensor.matmul(
                            phi[:, l, 128 * k:128 * (k + 1)],
                            phim[:, 4 * h + k, l, :], th_all[:, b, :],
                            start=True, stop=True)
                res = []
                v0 = None
                for l in range(u):
                    aphi = trigp.tile([128, FCOL], f32, tag="aphi", name=f"aphi{c}")
                    nc.scalar.activation(aphi[:], phi[:, l, :], ACT.Abs)
                    cs2 = trigp.tile([128, 2, FCOL], dt_st,
                                     tag="cs2", name=f"cs2{c}")
                    nc.scalar.activation(cs2[:, 1, :], phi[:, l, :], ACT.Sin,
                                         scale=0.5)
                    nc.scalar.activation(cs2[:, 0, :], aphi[:], ACT.Sin,
                                         bias=halfpi.ap(), scale=-0.5)
                    ssq = trigp.tile([128, FCOL], dt_st, tag="ssq", name=f"ssq{c}")
                    nc.vector.tensor_tensor(out=ssq[:], in0=cs2[:, 1, :],
                                            in1=cs2[:, 1, :], op=MULT)
                    cs1 = trigp.tile([128, 2, FCOL], dt_st,
                                     tag="cs1", name=f"cs1{c}")
                    # cos(phi) = 1 - 2 sin^2(phi/2)
                    nc.scalar.activation(cs1[:, 0, :], ssq[:], ACT.Copy,
                                         bias=1.0, scale=-2.0)
                    # sin(phi') = 2 sin(phi'/2) cos(phi'/2)
                    nc.vector.scalar_tensor_tensor(out=cs1[:, 1, :],
                                                   in0=cs2[:, 1, :],
                                                   scalar=2.0,
                                                   in1=cs2[:, 0, :],
                                                   op0=MULT, op1=MULT)
                    if l == lmap[0]:
                        v0 = trigp.tile([128, FCOL], dt_st,
                                        tag="v0", name=f"v0{c}")
                        nc.scalar.activation(v0[0:64, :], ssq[0:64, :],
                                             ACT.Copy, bias=1.0, scale=-2.0)
                        nc.vector.scalar_tensor_tensor(
                            out=v0[64:128, :], in0=cs2[64:128, 1, :],
                            scalar=2.0, in1=cs2[64:128, 0, :],
                            op0=MULT, op1=MULT)
                    res.append((cs1, cs2))
                return res, v0

            def dapply_pair(Ps, csts, split):
                """For each chain c: t1_c = cos (.) P_c, t2_c = sin (.) P_c.
                split=False: one broadcast multiply on Vector (reads PSUM
                twice).  split=True: downconvert P once on Scalar, then
                multiplies on Vector/GpSimd from fp16 SBUF."""
                outs = []
                if not split:
                    for c, (P, cst) in enumerate(zip(Ps, csts)):
                        tb = tmpp.tile([128, 2, FCOL], dt_st, tag=f"tb{c}")
                        prep = P[:].unsqueeze(1).to_broadcast((128, 2, FCOL))
                        nc.vector.tensor_tensor(out=tb[:], in0=csts[c][:],
                                                in1=prep, op=MULT)
                        outs.append((tb[:, 0, :], tb[:, 1, :]))
                    return outs
                Pcs = []
                for c, P in enumerate(Ps):
                    Pc = tmpp.tile([128, FCOL], dt_st, tag=f"pc{c}")
                    nc.scalar.activation(Pc[:], P[:], ACT.Copy, bias=0.0)
                    Pcs.append(Pc)
                t1s = []
                for c in range(2):
                    t1 = tmpp.tile([128, FCOL], dt_st, tag=f"t1{c}")
                    nc.vector.tensor_tensor(out=t1[:], in0=csts[c][:, 0, :],
                                            in1=Pcs[c][:], op=MULT)
                    t1s.append(t1)
                for c in range(2):
                    t2 = tmpp.tile([128, FCOL], dt_st, tag=f"t2{c}")
                    nc.gpsimd.tensor_tensor(out=t2[:], in0=csts[c][:, 1, :],
                                            in1=Pcs[c][:], op=MULT)
                    outs.append((t1s[c][:], t2[:]))
                return outs

            def gate_mm_pair(pool, tag, gi, tts):
                """P_c = W_gi . t1_c + W_gi_swapped . t2_c, W-adjacent."""
                Ps = [pool.tile([128, FCOL], f32, tag=tag,
                                name=f"{tag}_{gi}_{c}") for c in range(2)]
                for c in range(2):
                    nc.tensor.matmul(Ps[c][:], wm[:, 2 * gi, :], tts[c][0],
                                     start=True, stop=False)
                for c in range(2):
                    nc.tensor.matmul(Ps[c][:], wm[:, 2 * gi + 1, :],
                                     tts[c][1], start=False, stop=True)
                return Ps

            # ---- main loop: adjacent tile pairs, chains interleaved -----
            pair_list = [(2 * pr, 2 * pr + 1) for pr in range(npairs)]

            def emit_l0(trig_pair):
                """layer-0 mid-gate matmuls for a pair (v0 -> P1)."""
                P1s = []
                for c in range(2):
                    P1 = gpA.tile([128, FCOL], f32, tag="P1")
                    nc.tensor.matmul(P1[:], wm[:, 0, :], trig_pair[c][1][:],
                                     start=True, stop=True)
                    P1s.append(P1)
                return P1s

            trigs = [emit_phi_trig(pair_list[0][c], c) for c in range(2)]
            P1s_carry = emit_l0(trigs)
            for pr, ts in enumerate(pair_list):
                trig_next = None
                P1s_next = None
                P2s = [None, None]
                for l in range(N_LAYERS):
                    li = lmap[l]
                    if l == 0:
                        P1s = P1s_carry
                    else:
                        tts = dapply_pair(P2s, [trigs[c][0][li][0]
                                                for c in range(2)],
                                          split=False)
                        P1s = gate_mm_pair(gpA, "P1", 1, tts)
                    tts = dapply_pair(P1s, [trigs[c][0][li][1]
                                            for c in range(2)], split=True)
                    P2s = gate_mm_pair(gpB, "P2", 2 + l, tts)
                    if l == 0 and pr + 1 < npairs:
                        trig_next = [emit_phi_trig(pair_list[pr + 1][c], c)
                                     for c in range(2)]
                    # software-pipeline: start the NEXT pair's layer-0 while
                    # this pair still has a full layer to go, so TensorE has
                    # ready work during this pair's elementwise stages.
                    if l == 1 and trig_next is not None:
                        P1s_next = emit_l0(trig_next)
                # ---- finish: square, sign-reduce (state as stationary) --
                sqs = []
                for c in range(2):
                    sq = sqp.tile([128, FCOL], dt_st, tag=f"sq{c}",
                                  name=f"sq{c}")
                    nc.scalar.activation(sq[:], P2s[c][:], ACT.Square)
                    sqs.append(sq)
                zo = zop.tile([128, 2, 4, 16], f32, tag="zo")
                for k in range(4):
                    for c in range(2):
                        nc.tensor.matmul(zo[:, c, k, :],
                                         sqs[c][:, 128 * k:128 * (k + 1)],
                                         sgm2[:], start=True, stop=True)
                for c in range(2):
                    t = ts[c]
                    b, h = t // 2, t % 2
                    nc.scalar.copy(od_all[:, b, h, :, :], zo[:, c, :, :])
                if trig_next is not None:
                    trigs = trig_next
                    P1s_carry = P1s_next

            # ---- final store --------------------------------------------
            oflat = out_ap[:].rearrange("(p s) w -> p (s w)", p=128)
            nc.sync.dma_start(oflat,
                              od_all[:].rearrange("p a b c d -> p (a b c d)"))

    nc.compile()
    return nc


def _get_nc(bs, u, lmap, dt_state_name=None):
    key = (bs, u, tuple(lmap), dt_state_name or DT_STATE)
    if key not in _NC_CACHE:
        _NC_CACHE[key] = _build_nc(bs, u, lmap, dt_state_name)
    return _NC_CACHE[key]


def _np_dt(name):
    import ml_dtypes
    return {"float32": np.float32, "float32r": np.float32,
            "float16": np.float16, "bfloat16": ml_dtypes.bfloat16}[name]


def kernel(x, weights, scaling):
    from concourse.bass_utils import run_bass_kernel_spmd

    x = np.ascontiguousarray(np.asarray(x, dtype=np.float32))
    B = x.shape[0]
    consts, u, lmap = _host_tensors(weights, scaling,
                                    dt_state=_np_dt(DT_STATE))

    chunk = N_CORES * G4 * FCOL * 2
    Bp = ((B + chunk - 1) // chunk) * chunk
    if Bp != B:
        xp = np.zeros((Bp, x.shape[1]), dtype=np.float32)
        xp[:B] = x
        x = xp
    bs = Bp // N_CORES

    nc = _get_nc(bs, u, lmap)
    xs = x.reshape(N_CORES, bs, x.shape[1])
    in_maps = [dict(consts, x=np.ascontiguousarray(xs[i]))
               for i in range(N_CORES)]
    res = run_bass_kernel_spmd(nc, in_maps, core_ids=list(range(N_CORES)))
    out = np.concatenate([r["out"] for r in res.results], axis=0)
    return out[:B]


# revision 32
# speedup vs baseline: 1.2719x; 1.0084x over previous
"""Trainium2 Bass kernel for nn_DataReuploadingEncoder (4-qubit data
re-uploading circuit, B=1048576 samples, 8-core data parallel).

Complex-real ("L2") layout: state partition index p = 64*c + 16*g + j with
c in {re=0, im=1}, g sample-group, j state index; free dim = samples.  Each
fixed 16x16 complex gate is ONE 128x128 real stationary (4 diagonal 32x32
complex-real blocks [[Wr,-Wi],[Wi,Wr]]).

Per layer:  s = D1 s ; s = M_mid s ; s = D2 s ; s = M_l s   where the D's
are per-sample diagonals exp(i*phi).  Applying D then M is computed as
    P_next = M . (cos (.) P)  +  Msw . (sin (.) P)
two accumulating matmuls, where Msw is M with its input partition halves
swapped: this folds the re/im cross-term of the diagonal into the
stationary (compute lanes are partition-locked and cannot cross the re/im
halves).  The +/- signs of the sin terms are FOLDED INTO THE TRIG TILES:
the phase matmul constants negate phi on im rows (phi' = (-1)^c phi_j).

Layer-0's D1 acts on the uniform state: its output is the full-angle trig
tile directly (folded sign on im rows), compensated by a column-flipped
copy of the mid gate (W0b).  The finish uses the squared state as the
matmul STATIONARY and the sign matrix as moving operand, so the output
lands with samples in partitions (no output transposes).

Tiles are processed in PAIRS with instruction-level interleaving of the
two chains, so every engine queue alternates between two independent
dependency chains.

Sample mapping per core (bs = 131072):
  flat load: fl[p, n] = x[1024 p + n//4, n%4],  transpose+tanh ->
  th_all[r=4*sl+i, b, p] = tanh(x[1024 p + 32 b + sl, i])
  tile T (2048 samples): b = T//2, parity h = T%2;
  free col f = 128 k + p (k in 0..3);  group g:  sl = 16 h + 4 k + g.
"""

import numpy as np

N_QUBITS = 4
N_LAYERS = 3
DIM = 16
G4 = 4          # sample groups per tile (partition packing)
FCOL = 512      # samples per group per tile -> 2048 samples per tile
N_CORES = 8

# ----------------------------------------------------------------------------
# host-side constant construction
# ----------------------------------------------------------------------------


def _rz(t):
    return np.diag([np.exp(-0.5j * t), np.exp(0.5j * t)]).astype(np.complex128)


def _ry(t):
    c, s = np.cos(t / 2), np.sin(t / 2)
    return np.array([[c, -s], [s, c]], dtype=np.complex128)


def _rot(phi, theta, omega):
    return _rz(omega) @ _ry(theta) @ _rz(phi)


def _kron4(mats):
    out = mats[0]
    for m in mats[1:]:
        out = np.kron(out, m)
    return out


def _cnot_mat(c, t):
    P = np.zeros((DIM, DIM), dtype=np.complex128)
    for j in range(DIM):
        bc = (j >> (3 - c)) & 1
        jj = j ^ (1 << (3 - t)) if bc else j
        P[jj, j] = 1.0
    return P


def _bit(j, i):
    return (j >> (3 - i)) & 1


def _build_constants(weights, scaling):
    weights = np.asarray(weights, dtype=np.float64)
    scaling = np.asarray(scaling, dtype=np.float64)

    A = np.zeros((N_LAYERS, N_QUBITS, DIM))
    for l in range(N_LAYERS):
        for i in range(N_QUBITS):
            for j in range(DIM):
                sgn = 1.0 if _bit(j, i) else -1.0
                A[l, i, j] = sgn * np.pi * scaling[l, i] / 2.0

    # dedup identical scaling rows (harness uses all-ones -> u == 1)
    uniq = []
    lmap = []
    for l in range(N_LAYERS):
        for k, ku in enumerate(uniq):
            if np.array_equal(A[l], A[ku]):
                lmap.append(k)
                break
        else:
            uniq.append(l)
            lmap.append(len(uniq) - 1)
    A_u = A[uniq]  # [u, 4, 16]

    S = np.diag([1.0, 1.0j]).astype(np.complex128)
    H = np.array([[1, 1], [1, -1]], dtype=np.complex128) / np.sqrt(2.0)
    SH = S @ H
    HSd = H @ S.conj().T

    C = np.eye(DIM, dtype=np.complex128)
    for i in range(N_QUBITS):
        C = _cnot_mat(i, (i + 1) % N_QUBITS) @ C

    F_SH = _kron4([SH] * 4)
    F_HS = _kron4([HSd] * 4)
    R = [_kron4([_rot(*weights[l, i]) for i in range(N_QUBITS)])
         for l in range(N_LAYERS)]

    gates = [F_SH,
             0.25 * (F_HS @ C @ R[0]),
             F_HS @ C @ R[1],
             C @ R[2]]

    sign = np.zeros((DIM, N_QUBITS))
    for j in range(DIM):
        for w in range(N_QUBITS):
            sign[j, w] = 1.0 - 2.0 * _bit(j, w)

    return A_u, lmap, gates, sign


def _cplx_block(M):
    """complex 16x16 -> real 32x32 on (c,j) vectors: [[Wr,-Wi],[Wi,Wr]]."""
    Wr, Wi = np.real(M), np.imag(M)
    B = np.zeros((32, 32))
    B[:16, :16] = Wr
    B[:16, 16:] = -Wi
    B[16:, :16] = Wi
    B[16:, 16:] = Wr
    return B


def _host_tensors(weights, scaling, dt_state=np.float16):
    A_u, lmap, gates, sign = _build_constants(weights, scaling)
    u = A_u.shape[0]

    # phase-matmul stationaries: phim[4h+k][r=4*sl+i, l, P=64c+16g+j]
    #   = (-1)^c * A_u[l,i,j] * [sl == 16h+4k+g]   (sign folding on im rows)
    phim = np.zeros((8, 128, u, 128), dtype=np.float64)
    for h in range(2):
        for k in range(4):
            for g in range(G4):
                sl = 16 * h + 4 * k + g
                for i in range(N_QUBITS):
                    r = 4 * sl + i
                    for l in range(u):
                        for c in range(2):
                            sgnc = 1.0 if c == 0 else -1.0
                            base = 64 * c + 16 * g
                            phim[4 * h + k, r, l, base:base + 16] = \
                                sgnc * A_u[l, i]
    phim = phim.astype(dt_state)

    # gate stationaries: [W0b, W0, G1, G2, G3]; lhsT[p_in, 2*gi, p_out] =
    # block[p_out_local, p_in_local] replicated over the 4 groups; 2*gi+1
    # is the input-half-SWAPPED copy (rows 0:64 <-> 64:128).
    blocks = []
    B0 = _cplx_block(gates[0])
    B0b = B0.copy()
    B0b[:, 16:] *= -1.0   # compensate folded (-sin) im rows of v0
    blocks.append(B0b)
    blocks.append(B0)
    for gi in range(1, 4):
        blocks.append(_cplx_block(gates[gi]))

    wm = np.zeros((128, 10, 128), dtype=np.float64)
    for gi, B in enumerate(blocks):
        lt = np.zeros((128, 128))
        for g in range(G4):
            for ci in range(2):
                for j in range(DIM):
                    p_in = 64 * ci + 16 * g + j
                    for co in range(2):
                        for j2 in range(DIM):
                            p_out = 64 * co + 16 * g + j2
                            lt[p_in, p_out] = B[16 * co + j2, 16 * ci + j]
        wm[:, 2 * gi, :] = lt
        perm = np.arange(128) ^ 64
        wm[:, 2 * gi + 1, :] = lt[perm, :]
    wm = wm.astype(dt_state)

    # sign-reduce MOVING operand: sgm2[64c+16g+j, 4g+w] = sign[j, w]
    sgm2 = np.zeros((128, 16), dtype=np.float64)
    for c in range(2):
        for g in range(G4):
            for j in range(DIM):
                sgm2[64 * c + 16 * g + j, 4 * g:4 * g + 4] = sign[j]
    sgm2 = sgm2.astype(dt_state)

    id128 = np.eye(128, dtype=np.float32)

    return {"phimats": phim, "wmats": wm, "signmat": sgm2,
            "id128": id128}, u, lmap


# ----------------------------------------------------------------------------
# bass kernel
# ----------------------------------------------------------------------------

_NC_CACHE = {}

DT_STATE = "float16"


def _build_nc(bs, u, lmap, dt_state_name=None):
    import concourse.tile as tile
    from concourse import bacc, mybir
    from contextlib import ExitStack

    f32 = mybir.dt.float32
    dt_st = getattr(mybir.dt, dt_state_name or DT_STATE)
    ACT = mybir.ActivationFunctionType
    MULT = mybir.AluOpType.mult

    ntiles = bs // (G4 * FCOL)
    assert bs % (G4 * FCOL) == 0
    assert ntiles % 2 == 0
    npairs = ntiles // 2
    nflat = bs * N_QUBITS // 128          # elements per partition, flat load
    nblk = nflat // 128
    assert nblk == ntiles // 2
    assert u <= 2

    nc = bacc.Bacc("TRN2", target_bir_lowering=False, debug=False)
    x_ap = nc.dram_tensor("x", [bs, N_QUBITS], f32, kind="ExternalInput").ap()
    phim_ap = nc.dram_tensor("phimats", [8, 128, u, 128], dt_st,
                             kind="ExternalInput").ap()
    wm_ap = nc.dram_tensor("wmats", [128, 10, 128], dt_st,
                           kind="ExternalInput").ap()
    sg_ap = nc.dram_tensor("signmat", [128, 16], dt_st,
                           kind="ExternalInput").ap()
    id128_ap = nc.dram_tensor("id128", [128, 128], f32,
                              kind="ExternalInput").ap()
    out_ap = nc.dram_tensor("out", [bs, N_QUBITS], f32,
                            kind="ExternalOutput").ap()

    halfpi = nc.alloc_sbuf_tensor("halfpi", [128, 1], f32)
    nc.gpsimd.memset(halfpi.ap(), float(np.pi / 2))
    nc.all_engine_barrier()

    with tile.TileContext(nc) as tc:
        with ExitStack() as ctx:
            consts = ctx.enter_context(tc.tile_pool(name="consts", bufs=1))
            bigp = ctx.enter_context(tc.tile_pool(name="big", bufs=1))
            trigp = ctx.enter_context(tc.tile_pool(name="trig", bufs=4))
            tmpp = ctx.enter_context(tc.tile_pool(name="tmp", bufs=4))
            sqp = ctx.enter_context(tc.tile_pool(name="sq", bufs=2))
            # PSUM: P1 3x2KB + P2 2x2KB + phi 2x2KB + zo 2x0.25KB = 14.5KB
            phip = ctx.enter_context(tc.tile_pool(name="phip", bufs=2,
                                                  space="PSUM"))
            gpA = ctx.enter_context(tc.tile_pool(name="gpA", bufs=3,
                                                 space="PSUM"))
            gpB = ctx.enter_context(tc.tile_pool(name="gpB", bufs=2,
                                                 space="PSUM"))
            zop = ctx.enter_context(tc.tile_pool(name="zop", bufs=1,
                                                 space="PSUM"))

            # constants
            phim = consts.tile([128, 8, u, 128], dt_st)
            for k in range(8):
                nc.sync.dma_start(phim[:, k, :, :], phim_ap[k, :, :, :])
            wm = consts.tile([128, 10, 128], dt_st)
            nc.sync.dma_start(wm[:], wm_ap[:])
            sgm2 = consts.tile([128, 16], dt_st)
            nc.sync.dma_start(sgm2[:], sg_ap[:])
            id128 = consts.tile([128, 128], f32)
            nc.sync.dma_start(id128[:], id128_ap[:])

            # ---- input: contiguous load, transpose, tanh -----------------
            fl = bigp.tile([128, nflat], f32)
            xflat = x_ap[:].rearrange("(p s) i -> p (s i)", p=128)
            nc.sync.dma_start(fl[:], xflat)
            th_all = bigp.tile([128, nblk, 128], dt_st)
            for b in range(nblk):
                tp = gpA.tile([128, 128], f32, tag="P1")
                nc.tensor.transpose(tp[:], fl[:, 128 * b:128 * (b + 1)],
                                    id128[:])
                nc.scalar.activation(th_all[:, b, :], tp[:], ACT.Tanh)

            # od_all free dims: (b, h, k, g, w) -> col 128b+64h+16k+4g+w
            od_all = bigp.tile([128, nblk, 2, 4, 16], f32)

            def emit_phi_trig(t, c):
                """phase matmuls + trig for tile t (chain c): returns
                ([per-l (cs1, cs2)], v0); trig tiles [128, 2, FCOL] fp16
                (0=cos, 1=sin; sin sign-folded: negated on im rows)."""
                b, h = t // 2, t % 2
                phi = phip.tile([128, u, FCOL], f32, tag="phi")
                for l in range(u):
                    for k in range(4):
                        nc.tensor.matmul(
                            phi[:, l, 128 * k:128 * (k + 1)],
                            phim[:, 4 * h + k, l, :], th_all[:, b, :],
                            start=True, stop=True)
                res = []
                v0 = None
                for l in range(u):
                    aphi = trigp.tile([128, FCOL], f32, tag="aphi", name=f"aphi{c}")
                    nc.scalar.activation(aphi[:], phi[:, l, :], ACT.Abs)
                    cs2 = trigp.tile([128, 2, FCOL], dt_st,
                                     tag="cs2", name=f"cs2{c}")
                    nc.scalar.activation(cs2[:, 1, :], phi[:, l, :], ACT.Sin,
                                         scale=0.5)
                    nc.scalar.activation(cs2[:, 0, :], aphi[:], ACT.Sin,
                                         bias=halfpi.ap(), scale=-0.5)
                    ssq = trigp.tile([128, FCOL], dt_st, tag="ssq", name=f"ssq{c}")
                    nc.vector.tensor_tensor(out=ssq[:], in0=cs2[:, 1, :],
                                            in1=cs2[:, 1, :], op=MULT)
                    cs1 = trigp.tile([128, 2, FCOL], dt_st,
                                     tag="cs1", name=f"cs1{c}")
                    # cos(phi) = 1 - 2 sin^2(phi/2)
                    nc.scalar.activation(cs1[:, 0, :], ssq[:], ACT.Copy,
                                         bias=1.0, scale=-2.0)
                    # sin(phi') = 2 sin(phi'/2) cos(phi'/2)
                    nc.vector.scalar_tensor_tensor(out=cs1[:, 1, :],
                                                   in0=cs2[:, 1, :],
                                                   scalar=2.0,
                                                   in1=cs2[:, 0, :],
                                                   op0=MULT, op1=MULT)
                    if l == lmap[0]:
                        v0 = trigp.tile([128, FCOL], dt_st,
                                        tag="v0", name=f"v0{c}")
                        nc.scalar.activation(v0[0:64, :], ssq[0:64, :],
                                             ACT.Copy, bias=1.0, scale=-2.0)
                        nc.vector.scalar_tensor_tensor(
                            out=v0[64:128, :], in0=cs2[64:128, 1, :],
                            scalar=2.0, in1=cs2[64:128, 0, :],
                            op0=MULT, op1=MULT)
                    res.append((cs1, cs2))
                return res, v0

            def dapply_pair(Ps, csts, split):
                """For each chain c: t1_c = cos (.) P_c, t2_c = sin (.) P_c.
                split=False: one broadcast multiply on Vector (reads PSUM
                twice).  split=True: downconvert P once on Scalar, then
                multiplies on Vector/GpSimd from fp16 SBUF."""
                outs = []
                if not split:
                    for c, (P, cst) in enumerate(zip(Ps, csts)):
                        tb = tmpp.tile([128, 2, FCOL], dt_st, tag=f"tb{c}")
                        prep = P[:].unsqueeze(1).to_broadcast((128, 2, FCOL))
                        nc.vector.tensor_tensor(out=tb[:], in0=csts[c][:],
                                                in1=prep, op=MULT)
                        outs.append((tb[:, 0, :], tb[:, 1, :]))
                    return outs
                Pcs = []
                for c, P in enumerate(Ps):
                    Pc = tmpp.tile([128, FCOL], dt_st, tag=f"pc{c}")
                    if c == 0:
                        nc.scalar.activation(Pc[:], P[:], ACT.Copy, bias=0.0)
                    else:
                        nc.vector.tensor_copy(Pc[:], P[:])
                    Pcs.append(Pc)
                t1s = []
                for c in range(2):
                    t1 = tmpp.tile([128, FCOL], dt_st, tag=f"t1{c}")
                    nc.vector.tensor_tensor(out=t1[:], in0=csts[c][:, 0, :],
                                            in1=Pcs[c][:], op=MULT)
                    t1s.append(t1)
                for c in range(2):
                    t2 = tmpp.tile([128, FCOL], dt_st, tag=f"t2{c}")
                    nc.gpsimd.tensor_tensor(out=t2[:], in0=csts[c][:, 1, :],
                                            in1=Pcs[c][:], op=MULT)
                    outs.append((t1s[c][:], t2[:]))
                return outs

            def gate_mm_pair(pool, tag, gi, tts):
                """P_c = W_gi . t1_c + W_gi_swapped . t2_c, W-adjacent."""
                Ps = [pool.tile([128, FCOL], f32, tag=tag,
                                name=f"{tag}_{gi}_{c}") for c in range(2)]
                for c in range(2):
                    nc.tensor.matmul(Ps[c][:], wm[:, 2 * gi, :], tts[c][0],
                                     start=True, stop=False)
                for c in range(2):
                    nc.tensor.matmul(Ps[c][:], wm[:, 2 * gi + 1, :],
                                     tts[c][1], start=False, stop=True)
                return Ps

            # ---- main loop: adjacent tile pairs, chains interleaved -----
            pair_list = [(2 * pr, 2 * pr + 1) for pr in range(npairs)]

            def emit_l0(trig_pair):
                """layer-0 mid-gate matmuls for a pair (v0 -> P1)."""
                P1s = []
                for c in range(2):
                    P1 = gpA.tile([128, FCOL], f32, tag="P1")
                    nc.tensor.matmul(P1[:], wm[:, 0, :], trig_pair[c][1][:],
                                     start=True, stop=True)
                    P1s.append(P1)
                return P1s

            trigs = [emit_phi_trig(pair_list[0][c], c) for c in range(2)]
            P1s_carry = emit_l0(trigs)
            for pr, ts in enumerate(pair_list):
                trig_next = None
                P1s_next = None
                P2s = [None, None]
                for l in range(N_LAYERS):
                    li = lmap[l]
                    if l == 0:
                        P1s = P1s_carry
                    else:
                        tts = dapply_pair(P2s, [trigs[c][0][li][0]
                                                for c in range(2)],
                                          split=False)
                        P1s = gate_mm_pair(gpA, "P1", 1, tts)
                    tts = dapply_pair(P1s, [trigs[c][0][li][1]
                                            for c in range(2)], split=True)
                    P2s = gate_mm_pair(gpB, "P2", 2 + l, tts)
                    if l == 0 and pr + 1 < npairs:
                        trig_next = [emit_phi_trig(pair_list[pr + 1][c], c)
                                     for c in range(2)]
                    # software-pipeline: start the NEXT pair's layer-0 while
                    # this pair still has a full layer to go, so TensorE has
                    # ready work during this pair's elementwise stages.
                    if l == 1 and trig_next is not None:
                        P1s_next = emit_l0(trig_next)
                # ---- finish: square, sign-reduce (state as stationary) --
                sqs = []
                for c in range(2):
                    sq = sqp.tile([128, FCOL], dt_st, tag=f"sq{c}",
                                  name=f"sq{c}")
                    nc.scalar.activation(sq[:], P2s[c][:], ACT.Square)
                    sqs.append(sq)
                zo = zop.tile([128, 2, 4, 16], f32, tag="zo")
                for k in range(4):
                    for c in range(2):
                        nc.tensor.matmul(zo[:, c, k, :],
                                         sqs[c][:, 128 * k:128 * (k + 1)],
                                         sgm2[:], start=True, stop=True)
                for c in range(2):
                    t = ts[c]
                    b, h = t // 2, t % 2
                    nc.scalar.copy(od_all[:, b, h, :, :], zo[:, c, :, :])
                if trig_next is not None:
                    trigs = trig_next
                    P1s_carry = P1s_next

            # ---- final store --------------------------------------------
            oflat = out_ap[:].rearrange("(p s) w -> p (s w)", p=128)
            nc.sync.dma_start(oflat,
                              od_all[:].rearrange("p a b c d -> p (a b c d)"))

    nc.compile()
    return nc


def _get_nc(bs, u, lmap, dt_state_name=None):
    key = (bs, u, tuple(lmap), dt_state_name or DT_STATE)
    if key not in _NC_CACHE:
        _NC_CACHE[key] = _build_nc(bs, u, lmap, dt_state_name)
    return _NC_CACHE[key]


def _np_dt(name):
    import ml_dtypes
    return {"float32": np.float32, "float32r": np.float32,
            "float16": np.float16, "bfloat16": ml_dtypes.bfloat16}[name]


def kernel(x, weights, scaling):
    from concourse.bass_utils import run_bass_kernel_spmd

    x = np.ascontiguousarray(np.asarray(x, dtype=np.float32))
    B = x.shape[0]
    consts, u, lmap = _host_tensors(weights, scaling,
                                    dt_state=_np_dt(DT_STATE))

    chunk = N_CORES * G4 * FCOL * 2
    Bp = ((B + chunk - 1) // chunk) * chunk
    if Bp != B:
        xp = np.zeros((Bp, x.shape[1]), dtype=np.float32)
        xp[:B] = x
        x = xp
    bs = Bp // N_CORES

    nc = _get_nc(bs, u, lmap)
    xs = x.reshape(N_CORES, bs, x.shape[1])
    in_maps = [dict(consts, x=np.ascontiguousarray(xs[i]))
               for i in range(N_CORES)]
    res = run_bass_kernel_spmd(nc, in_maps, core_ids=list(range(N_CORES)))
    out = np.concatenate([r["out"] for r in res.results], axis=0)
    return out[:B]
